# revision 10
# baseline (speedup 1.0000x reference)
"""Trainium2 Bass kernel for nn_DSWNV_84387517432212 (WaveNet-vocoder dense CNN).

Sharding: 8 cores = 4 batches x 2 asymmetric time-halves. Both halves run the
same program over EXT=4480 columns; half A covers output t in [0, 4420) with
no left halo (t<0 region never computed), half B covers t in [4420, 8249)
with a 650-sample halo occupying its first 650 columns. The host keeps the
valid slice from each.

Device algorithm (per core), all SBUF tensors bf16 (PSUM fp32):
  B) causal conv (k=6) + softsign -> h0 over EXT, emitted first; audio DMA is
     split so the first block starts early.
  A) conditioning: scale_in -> aux0 -> aux1 convs on 75 frames (fp32), then
     yaT[l] = [a2.T @ inx_w[l].T ; cb]; the x-upsample + inx 1x1 conv
     collapses into one K=76 matmul against a host-built selector matrix.
     Runs in the shadow of phase B.
  C) 9 gated dilated-conv layers over 5 chunks of 896 cols (2 subtiles of
     448). h channels 128..191 stored twice (upper half shifted by the next
     layer's dilation) so the k=6 dilated conv runs as 9 full-K chunks.
     The tanh half of the mixed z/t PSUM block is realigned to partitions
     0..63 with a selector matmul (no SBUF-SBUF DMA on the recursion).
     Skip contributions accumulate directly in PSUM across all 9 layers.
     Chunk tails stay in SBUF (GPSIMD copies), no DRAM round trip.
  D) relu -> out1 1x1 -> relu -> out2 computed transposed per chunk; output
     DMAs out in bf16, host converts to f32.
"""

import os

import numpy as np

import concourse.mybir as mybir
import concourse.tile as tile
from concourse import bacc
from concourse.bass_utils import run_bass_kernel_spmd

F32 = mybir.dt.float32
BF16 = mybir.dt.bfloat16
AF = mybir.ActivationFunctionType
ALU = mybir.AluOpType

NPBF = mybir.dt.np(BF16)

# model dims
B, T, NQ, HID, KK, UP, TAUX, NCOND = 4, 8249, 256, 192, 6, 110, 75, 486
DILS = [1, 6, 36, 1, 6, 36, 1, 6, 36]
# sharding / tiling
EXT = 4480           # columns computed per core
TA = 4420            # half A valid outputs [0, TA); half B outputs [TA, T)
HALO = 650           # receptive field (half B warm-up columns)
APAD = 5             # causal conv left taps
TAIL = 180           # max layer lookback 5*36
NS = 448             # matmul subtile
CHW = 896            # chunk width (2 subtiles)
NCH = EXT // CHW     # 5 chunks
NB = 448             # causal conv block

# xh channel permutation: [z 0:128 | t 0:128 | z 128:192 | t 128:192]
PERM = list(range(0, 128)) + list(range(192, 320)) + \
    list(range(128, 192)) + list(range(320, 384))


def _bf(x):
    return np.ascontiguousarray(np.asarray(x, np.float32).astype(NPBF))


def _pack_weights(inp):
    """Host-side weight packing into SBUF-layout arrays."""
    w = {}
    causal_w = inp["causal_w"]          # (192, 256, 6)
    w["wc"] = _bf(np.stack(
        [causal_w[:, rc * 128:(rc + 1) * 128, tap].T
         for tap in range(6) for rc in range(2)], axis=1))          # (128,12,192)
    bc = np.zeros((128, 2), np.float32)
    bc[:, 0] = inp["causal_b"][0:128]
    bc[0:64, 1] = inp["causal_b"][128:192]
    w["bcp"] = bc                                                   # (128,2) f32

    dil = inp["dilh_w"][:, PERM, :, :]  # (9, 384, 192, 6)
    w["wd01"] = _bf(dil[:, :, 0:128, :].transpose(0, 2, 3, 1))      # (9,128,6,384)
    d2 = dil[:, :, 128:192, :].transpose(0, 2, 3, 1)                # (9,64,6,384)
    w["wd2p"] = _bf(np.concatenate(
        [d2[:, :, 0::2, :], d2[:, :, 1::2, :]], axis=1))            # (9,128,3,384)
    bd = inp["dilh_b"][:, PERM]         # (9, 384)
    w["bd"] = np.ascontiguousarray(
        bd.reshape(9, 3, 128).transpose(2, 0, 1)).astype(np.float32)  # (128,9,3)

    sk = inp["skip_w"][:, :, :, 0]      # (9, 256, 192)
    w["ws01"] = _bf(sk[:, :, 0:128].transpose(2, 0, 1))             # (128,9,256)
    w["ws2"] = _bf(sk[:, :, 128:192].transpose(2, 0, 1))            # (64,9,256)
    w["bss"] = np.ascontiguousarray(
        inp["skip_b"].sum(0).reshape(2, 128).T).astype(np.float32)  # (128,2)

    w["wsc"] = np.ascontiguousarray(inp["scale_in_w"][:, :, 0].T)   # (54,54) f32
    w["bsc"] = inp["scale_in_b"].reshape(54, 1).astype(np.float32)
    w["wa0"] = np.ascontiguousarray(
        inp["aux0_w"].transpose(1, 2, 0)).astype(np.float32)        # (54,3,162)
    b0 = np.zeros((128, 2), np.float32)
    b0.T.flat[:162] = inp["aux0_b"]
    w["ba0"] = b0
    a1t = inp["aux1_w"].transpose(1, 2, 0)                          # (162,3,486)
    w["wa1a"] = np.ascontiguousarray(a1t[0:128]).astype(np.float32)
    w["wa1b"] = np.ascontiguousarray(a1t[128:162]).astype(np.float32)
    b1 = np.zeros((128, 4), np.float32)
    b1.T.flat[:486] = inp["aux1_b"]
    w["ba1"] = b1

    inx = inp["inx_w"][:, :, :, 0][:, PERM, :]   # (9, 384, 486)
    wi = np.zeros((9, 4, 128, 384), np.float32)
    for r in range(4):
        n = min(128, 486 - r * 128)
        wi[:, r, :n, :] = inx[:, :, r * 128:r * 128 + n].transpose(0, 2, 1)
    w["wi"] = _bf(wi)
    w["cb"] = _bf((inp["up_b"] * inx.sum(2) + inp["inx_b"][:, PERM])
                  .reshape(9, 1, 384))                              # (9,1,384)

    o1 = inp["out1_w"][:, :, 0]         # (256, 256)
    w["wo1"] = _bf(o1.T.reshape(2, 128, 256).transpose(1, 0, 2))    # (128,2,256)
    w["bo1"] = np.ascontiguousarray(
        inp["out1_b"].reshape(2, 128).T).astype(np.float32)         # (128,2)
    o2 = inp["out2_w"][:, :, 0]
    w["wo2"] = _bf(o2.T.reshape(2, 128, 256).transpose(1, 0, 2))    # (128,2,256)
    w["bo2row"] = _bf(inp["out2_b"].reshape(1, 256))
    eye = np.zeros((128, 64), np.float32)
    eye[np.arange(64) + 64, np.arange(64)] = 1.0
    w["eye"] = _bf(eye)
    w["onesr"] = _bf(np.ones((1, 128)))
    return w


def _per_core_arrays(inp, w, b, half):
    """Per-core input map (audio shard, selector, aux) + shared weights."""
    t0 = 0 if half == 0 else TA - HALO
    audio = np.zeros((NQ, APAD + EXT), np.float32)
    g0 = t0 - APAD
    s0, s1 = max(0, g0), min(T, g0 + APAD + EXT)
    audio[:, s0 - g0:s1 - g0] = inp["audio"][b, :, s0:s1]

    # selector: rows 0..74 = aux frames scaled by up_w, row 75 = validity
    sel = np.zeros((76, EXT), np.float32)
    t = t0 + np.arange(EXT)
    valid = (t >= 0) & (t < T)
    tv = t[valid]
    sel[(tv + 1) // UP, np.where(valid)[0]] = inp["up_w"][(tv + 1) % UP]
    sel[75, valid] = 1.0

    m = {
        "audio_in": _bf(audio),
        "sel_in": _bf(sel),
        "aux_in": np.ascontiguousarray(inp["aux"][b]).astype(np.float32),
    }
    for k, v in w.items():
        m[k + "_in"] = v
    return m


def build_kernel():
    nc = bacc.Bacc(None, target_bir_lowering=False)
    d = {}
    shapes = {
        "audio_in": (BF16, (NQ, APAD + EXT)), "sel_in": (BF16, (76, EXT)),
        "aux_in": (F32, (54, TAUX)),
        "wc_in": (BF16, (128, 12, HID)), "bcp_in": (F32, (128, 2)),
        "wd01_in": (BF16, (9, 128, 6, 384)), "wd2p_in": (BF16, (9, 128, 3, 384)),
        "bd_in": (F32, (128, 9, 3)),
        "ws01_in": (BF16, (128, 9, 256)), "ws2_in": (BF16, (64, 9, 256)),
        "bss_in": (F32, (128, 2)),
        "wsc_in": (F32, (54, 54)), "bsc_in": (F32, (54, 1)),
        "wa0_in": (F32, (54, 3, 162)), "ba0_in": (F32, (128, 2)),
        "wa1a_in": (F32, (128, 3, NCOND)), "wa1b_in": (F32, (34, 3, NCOND)),
        "ba1_in": (F32, (128, 4)),
        "wi_in": (BF16, (9, 4, 128, 384)), "cb_in": (BF16, (9, 1, 384)),
        "wo1_in": (BF16, (128, 2, 256)), "bo1_in": (F32, (128, 2)),
        "wo2_in": (BF16, (128, 2, 256)), "bo2row_in": (BF16, (1, 256)),
        "eye_in": (BF16, (128, 64)), "onesr_in": (BF16, (1, 128)),
    }
    for k, (dt, shp) in shapes.items():
        d[k] = nc.dram_tensor(k, list(shp), dt, kind="ExternalInput")
    y_d = nc.dram_tensor("y", [EXT, NQ], BF16, kind="ExternalOutput")

    mm = nc.tensor.matmul

    import contextlib

    with tile.TileContext(nc) as tc:
        with tc.tile_pool(name="res", bufs=1) as res:
            sel_sb = res.tile([76, EXT], BF16)
            nc.sync.dma_start(out=sel_sb, in_=d["sel_in"][:, :])
            h0_01 = res.tile([128, TAIL + EXT], BF16)
            h0_2 = res.tile([128, TAIL + EXT], BF16)
            for t_ in (h0_01, h0_2):
                nc.gpsimd.memset(t_[:, 0:TAIL], 0.0)
            # per-layer SBUF tail buffers (prev-layer h for next chunk)
            tl01 = [res.tile([128, TAIL], BF16, name=f"tl01_{ll}")
                    for ll in range(9)]
            tl2 = [res.tile([128, TAIL], BF16, name=f"tl2_{ll}")
                   for ll in range(9)]
            for t_ in tl01 + tl2:
                nc.gpsimd.memset(t_, 0.0)
            yaT = [res.tile([76, 384], BF16, name=f"yaT{ll}") for ll in range(9)]
            ws01 = res.tile([128, 9, 256], BF16)
            nc.sync.dma_start(out=ws01, in_=d["ws01_in"][:, :, :])
            ws2 = res.tile([64, 9, 256], BF16)
            nc.sync.dma_start(out=ws2, in_=d["ws2_in"][:, :, :])
            bd_sb = res.tile([128, 9, 3], F32)
            nc.sync.dma_start(out=bd_sb, in_=d["bd_in"][:, :, :])
            bss_sb = res.tile([128, 2], F32)
            nc.sync.dma_start(out=bss_sb, in_=d["bss_in"][:, :])
            bo1_sb = res.tile([128, 2], F32)
            nc.sync.dma_start(out=bo1_sb, in_=d["bo1_in"][:, :])
            wo1 = res.tile([128, 2, 256], BF16)
            nc.sync.dma_start(out=wo1, in_=d["wo1_in"][:, :, :])
            wo2 = res.tile([128, 2, 256], BF16)
            nc.sync.dma_start(out=wo2, in_=d["wo2_in"][:, :, :])
            bo2row = res.tile([1, 256], BF16)
            nc.sync.dma_start(out=bo2row, in_=d["bo2row_in"][:, :])
            eye_sb = res.tile([128, 64], BF16)
            nc.sync.dma_start(out=eye_sb, in_=d["eye_in"][:, :])
            onesr = res.tile([1, 128], BF16)
            nc.sync.dma_start(out=onesr, in_=d["onesr_in"][:, :])

            # ---------------- Phase A: conditioning ----------------
            with tc.tile_pool(name="ca", bufs=1) as ca, \
                 tc.tile_pool(name="cw", bufs=3) as cw, \
                 tc.tile_pool(name="cp", bufs=1, space="PSUM") as cp:
                aux_sb = ca.tile([54, TAUX], F32)
                nc.sync.dma_start(out=aux_sb, in_=d["aux_in"][:, :])
                wsc = ca.tile([54, 54], F32)
                nc.sync.dma_start(out=wsc, in_=d["wsc_in"][:, :])
                bsc = ca.tile([54, 1], F32)
                nc.sync.dma_start(out=bsc, in_=d["bsc_in"][:, :])
                ba0 = ca.tile([128, 2], F32)
                nc.sync.dma_start(out=ba0, in_=d["ba0_in"][:, :])
                ba1 = ca.tile([128, 4], F32)
                nc.sync.dma_start(out=ba1, in_=d["ba1_in"][:, :])
                wa0 = ca.tile([54, 3, 162], F32)
                nc.sync.dma_start(out=wa0, in_=d["wa0_in"][:, :, :])
                wa1a = ca.tile([128, 3, NCOND], F32)
                nc.sync.dma_start(out=wa1a, in_=d["wa1a_in"][:, :, :])
                wa1b = ca.tile([34, 3, NCOND], F32)
                nc.sync.dma_start(out=wa1b, in_=d["wa1b_in"][:, :, :])

                a0p = cp.tile([54, TAUX], F32, padded_shape=[128, TAUX])
                mm(a0p, wsc, aux_sb, start=True, stop=True)
                a0 = ca.tile([54, TAUX], F32)
                nc.scalar.activation(out=a0, in_=a0p, func=AF.Identity,
                                     bias=bsc)

                # aux0: k=3, dil=1, same-pad via partial-range accumulation
                a1blk = [(0, 128), (128, 34)]
                a1 = [ca.tile([wd, TAUX], F32, name=f"a1_{i}")
                      for i, (o0, wd) in enumerate(a1blk)]
                for i, (o0, wd) in enumerate(a1blk):
                    a1p = cp.tile([wd, TAUX], F32, name=f"a1p{i}", tag="a1p",
                                  bufs=2, padded_shape=[128, TAUX])
                    ls = wa0[:, :, o0:o0 + wd]
                    mm(a1p, ls[:, 1, :], a0, start=True, stop=False)
                    mm(a1p[:, 1:TAUX], ls[:, 0, :], a0[:, 0:TAUX - 1],
                       start=False, stop=False)
                    mm(a1p[:, 0:TAUX - 1], ls[:, 2, :], a0[:, 1:TAUX],
                       start=False, stop=True)
                    nc.scalar.activation(out=a1[i], in_=a1p, func=AF.Identity,
                                         bias=ba0[0:wd, i:i + 1])

                # aux1: k=3, dil=3, same-pad
                a2blk = [(0, 128), (128, 128), (256, 128), (384, 102)]
                a2 = [ca.tile([wd, TAUX], BF16, name=f"a2_{i}")
                      for i, (o0, wd) in enumerate(a2blk)]
                for i, (o0, wd) in enumerate(a2blk):
                    a2p = cp.tile([wd, TAUX], F32, name=f"a2p{i}", tag="a2p",
                                  bufs=2, padded_shape=[128, TAUX])
                    for kc, wsrc in enumerate([wa1a, wa1b]):
                        ls = wsrc[:, :, o0:o0 + wd]
                        rhs = a1[kc]
                        mm(a2p, ls[:, 1, :], rhs, start=(kc == 0), stop=False)
                        mm(a2p[:, 3:TAUX], ls[:, 0, :], rhs[:, 0:TAUX - 3],
                           start=False, stop=False)
                        mm(a2p[:, 0:TAUX - 3], ls[:, 2, :], rhs[:, 3:TAUX],
                           start=False, stop=(kc == 1))
                    nc.scalar.activation(out=a2[i], in_=a2p, func=AF.Identity,
                                         bias=ba1[0:wd, i:i + 1])

                # yaT[l] rows 0..74 = a2.T @ inx_w[l].T ; row 75 = cb
                for ll in range(9):
                    wi_sb = cw.tile([128, 4, 384], BF16, tag="wi")
                    nc.sync.dma_start(
                        out=wi_sb,
                        in_=d["wi_in"][ll, :, :, :].rearrange("r p n -> p r n"))
                    yp = cp.tile([TAUX, 384], F32, tag="yp", bufs=2,
                                 padded_shape=[128, 384])
                    for r, (o0, wd) in enumerate(a2blk):
                        mm(yp, a2[r], wi_sb[0:wd, r, :], start=(r == 0),
                           stop=(r == 3))
                    nc.scalar.activation(out=yaT[ll][0:TAUX, :], in_=yp,
                                         func=AF.Copy)
                    nc.sync.dma_start(out=yaT[ll][TAUX:76, :],
                                      in_=d["cb_in"][ll, :, :])

            # -------- Phase B (causal conv + softsign) woven into Phase C ----
            with tc.tile_pool(name="pb", bufs=1) as pb, \
                 tc.tile_pool(name="pbs", bufs=2) as pbs, \
                 tc.tile_pool(name="hw", bufs=2) as hw, \
                 tc.tile_pool(name="wdp", bufs=3) as wdp, \
                 tc.tile_pool(name="scr", bufs=2) as scr, \
                 tc.tile_pool(name="od", bufs=1) as od, \
                 tc.tile_pool(name="pc", bufs=1, space="PSUM") as pc:
                a_t = [pb.tile([128, APAD + EXT], BF16, name=f"aud{i}")
                       for i in range(2)]
                NSPL = 4
                spl = (APAD + EXT) // NSPL
                for i in range(2):
                    src = d["audio_in"][i * 128:(i + 1) * 128, :]
                    for s in range(NSPL):
                        cb0, cb1 = s * spl, (s + 1) * spl if s < NSPL - 1 \
                            else APAD + EXT
                        nc.sync.dma_start(out=a_t[i][:, cb0:cb1],
                                          in_=src[:, cb0:cb1])
                wc = pb.tile([128, 12, HID], BF16)
                nc.sync.dma_start(out=wc, in_=d["wc_in"][:, :, :])
                bcp = pb.tile([128, 2], F32)
                nc.sync.dma_start(out=bcp, in_=d["bcp_in"][:, :])

                cblk = [(0, 128), (128, 64)]

                def emit_b_block(st):
                    ccp = [pc.tile([wd, NB], F32, tag="hc",
                                   name=f"ccp{st}_{i}", bufs=2,
                                   padded_shape=[128, NB])
                           for i, (o0, wd) in enumerate(cblk)]
                    for i, (o0, wd) in enumerate(cblk):
                        for tap in range(6):
                            for rc in range(2):
                                mm(ccp[i],
                                   wc[:, tap * 2 + rc, o0:o0 + wd],
                                   a_t[rc][:, st * NB + tap:st * NB + tap + NB],
                                   start=(tap == 0 and rc == 0),
                                   stop=(tap == 5 and rc == 1))
                    for i, (o0, wd) in enumerate(cblk):
                        den = pbs.tile([wd, NB], F32, tag=f"ab{i}",
                                       padded_shape=[128, NB])
                        nc.scalar.activation(out=den, in_=ccp[i], func=AF.Abs,
                                             bias=bcp[0:wd, i:i + 1])
                        nc.vector.tensor_scalar(
                            out=den, in0=den, scalar1=1.0, scalar2=None,
                            op0=ALU.add)
                        rr = pbs.tile([wd, NB], F32, tag=f"rr{i}",
                                      padded_shape=[128, NB])
                        nc.vector.reciprocal_approx_fast(out=rr, in_=den)
                        dst = h0_01 if i == 0 else h0_2
                        nc.vector.scalar_tensor_tensor(
                            out=dst[0:wd, TAIL + st * NB:TAIL + (st + 1) * NB],
                            in0=ccp[i], scalar=bcp[0:wd, i:i + 1], in1=rr,
                            op0=ALU.add, op1=ALU.mult)
                    # shifted upper copy of h0 ch 128..191 (layer 0 dil = 1)
                    nc.sync.dma_start(
                        out=h0_2[64:128,
                                 TAIL + st * NB - 1:TAIL + (st + 1) * NB - 1],
                        in_=h0_2[0:64,
                                 TAIL + st * NB:TAIL + (st + 1) * NB])

                nb_total = EXT // NB
                for st in range(3):
                    emit_b_block(st)
                nb_next = 3

                for c in range(NCH):
                    # weave remaining causal-conv blocks ahead of the chunks
                    # that consume them (chunk c+1 needs h0 through block 2c+3)
                    while nb_next < min(nb_total, 2 * c + 5):
                        emit_b_block(nb_next)
                        nb_next += 1
                    c0 = c * CHW
                    skp = [pc.tile([128, NS], F32, tag=f"sk{ob}_{st}", bufs=1,
                                   name=f"skp{c}_{ob}_{st}")
                           for ob in range(2) for st in range(2)]
                    prev01, prev2, poff = h0_01, h0_2, TAIL + c0

                    for ll in range(9):
                        dil = DILS[ll]
                        dnx = DILS[ll + 1] if ll < 8 else 1
                        cur01 = hw.tile([128, TAIL + CHW], BF16, tag="h01")
                        cur2 = hw.tile([128, TAIL + CHW], BF16, tag="h2")
                        nc.gpsimd.tensor_copy(out=cur01[:, 0:TAIL],
                                              in_=tl01[ll])
                        nc.gpsimd.tensor_copy(out=cur2[:, 0:TAIL],
                                              in_=tl2[ll])
                        wd01 = wdp.tile([128, 6, 384], BF16, tag="wd01")
                        wd2p = wdp.tile([128, 3, 384], BF16, tag="wd2p")
                        nc.sync.dma_start(out=wd01,
                                          in_=d["wd01_in"][ll, :, :, :])
                        nc.sync.dma_start(out=wd2p,
                                          in_=d["wd2p_in"][ll, :, :, :])

                        for st in range(CHW // NS):
                            sb_ = c0 + st * NS       # col in sel
                            lb = poff + st * NS      # col in prev buffers
                            xh = []
                            for mb in range(3):
                                xcp = pc.tile([128, NS], F32, tag="xc",
                                              bufs=2, name=f"xcp{mb}")
                                mm(xcp, yaT[ll][:, mb * 128:(mb + 1) * 128],
                                   sel_sb[:, sb_:sb_ + NS], start=True,
                                   stop=True)
                                xc_sb = scr.tile([128, NS], BF16,
                                                 tag=f"xcs{mb}")
                                nc.scalar.activation(out=xc_sb, in_=xcp,
                                                     func=AF.Copy)
                                hcp = pc.tile([128, NS], F32, tag="hc",
                                              bufs=2, name=f"hcp{mb}")
                                for tap in range(6):
                                    off = (tap - 5) * dil
                                    mm(hcp,
                                       wd01[:, tap, mb * 128:(mb + 1) * 128],
                                       prev01[:, lb + off:lb + off + NS],
                                       start=(tap == 0), stop=False)
                                for j in range(3):
                                    off = (2 * j - 5) * dil
                                    mm(hcp,
                                       wd2p[:, j, mb * 128:(mb + 1) * 128],
                                       prev2[:, lb + off:lb + off + NS],
                                       start=False, stop=(j == 2))
                                xh_sb = scr.tile([128, NS], BF16,
                                                 tag=f"xh{mb}")
                                nc.vector.scalar_tensor_tensor(
                                    out=xh_sb, in0=hcp,
                                    scalar=bd_sb[:, ll, mb:mb + 1],
                                    in1=xc_sb, op0=ALU.add, op1=ALU.mult)
                                xh.append(xh_sb)
                            # xh blocks (permuted): [z0:128], [t0:128],
                            #                       [z128:192 | t128:192]
                            # realign t128:192 to partitions 0..63 via PE
                            shp = pc.tile([64, NS], F32, tag="xc", bufs=2,
                                          name="shp", padded_shape=[128, NS])
                            mm(shp, eye_sb, xh[2], start=True, stop=True)
                            xh2b = scr.tile([64, NS], BF16, tag="xh2b",
                                            padded_shape=[128, NS])
                            nc.scalar.activation(out=xh2b, in_=shp,
                                                 func=AF.Tanh)
                            nc.scalar.activation(out=xh[0], in_=xh[0],
                                                 func=AF.Sigmoid)
                            nc.scalar.activation(out=xh[2][0:64, :],
                                                 in_=xh[2][0:64, :],
                                                 func=AF.Sigmoid)
                            nc.scalar.activation(out=xh[1], in_=xh[1],
                                                 func=AF.Tanh)
                            wcol = TAIL + st * NS
                            for i, (zz, ttt, hp, cdst, wd_) in enumerate([
                                    (xh[0], xh[1], prev01[:, lb:lb + NS],
                                     cur01, 128),
                                    (xh[2][0:64, :], xh2b,
                                     prev2[0:64, lb:lb + NS], cur2, 64)]):
                                dd = scr.tile([wd_, NS], BF16, tag=f"dd{i}",
                                              padded_shape=[128, NS])
                                nc.vector.tensor_tensor(
                                    out=dd, in0=hp, in1=ttt, op=ALU.subtract)
                                nc.vector.tensor_tensor(
                                    out=dd, in0=zz, in1=dd, op=ALU.mult)
                                nc.vector.tensor_tensor(
                                    out=cdst[0:wd_, wcol:wcol + NS],
                                    in0=ttt, in1=dd, op=ALU.add)
                            # shifted upper copy for next layer's pair chunks
                            nc.sync.dma_start(
                                out=cur2[64:128, wcol - dnx:wcol - dnx + NS],
                                in_=cur2[0:64, wcol:wcol + NS])
                            # skip conv accumulates in PSUM across layers
                            for ob in range(2):
                                mm(skp[ob * 2 + st],
                                   ws01[:, ll, ob * 128:(ob + 1) * 128],
                                   cur01[:, wcol:wcol + NS],
                                   start=(ll == 0), stop=False,
                                   skip_group_check=True)
                                mm(skp[ob * 2 + st],
                                   ws2[:, ll, ob * 128:(ob + 1) * 128],
                                   cur2[0:64, wcol:wcol + NS],
                                   start=False, stop=(ll == 8),
                                   skip_group_check=True)
                        # save tails for next chunk (SBUF, via GPSIMD)
                        if c < NCH - 1:
                            nc.gpsimd.tensor_copy(
                                out=tl01[ll], in_=cur01[:, CHW:CHW + TAIL])
                            nc.gpsimd.tensor_copy(
                                out=tl2[ll], in_=cur2[:, CHW:CHW + TAIL])
                        prev01, prev2, poff = cur01, cur2, TAIL

                    # ---------------- Phase D: output convs ----------------
                    r1 = [od.tile([128, CHW], BF16, tag=f"r1{i}",
                                  name=f"r1{c}_{i}") for i in range(2)]
                    for st in range(2):
                        rlq = []
                        for kc in range(2):
                            rt = scr.tile([128, NS], BF16, tag=f"rlq{kc}")
                            nc.scalar.activation(
                                out=rt, in_=skp[kc * 2 + st], func=AF.Relu,
                                bias=bss_sb[:, kc:kc + 1])
                            rlq.append(rt)
                        q0 = st * NS
                        for ob in range(2):
                            o1p = pc.tile([128, NS], F32, tag="hc", bufs=2,
                                          name=f"o1p{ob}")
                            for kc in range(2):
                                mm(o1p,
                                   wo1[:, kc, ob * 128:(ob + 1) * 128],
                                   rlq[kc], start=(kc == 0), stop=(kc == 1))
                            nc.scalar.activation(
                                out=r1[ob][:, q0:q0 + NS], in_=o1p,
                                func=AF.Relu, bias=bo1_sb[:, ob:ob + 1])
                    for q0 in range(0, CHW, 128):
                        o2p = pc.tile([128, 256], F32, tag="xc", bufs=2,
                                      name="o2p")
                        mm(o2p, r1[0][:, q0:q0 + 128], wo2[:, 0, :],
                           start=True, stop=False)
                        mm(o2p, r1[1][:, q0:q0 + 128], wo2[:, 1, :],
                           start=False, stop=False)
                        mm(o2p, onesr[:, 0:128], bo2row,
                           start=False, stop=True)
                        og = od.tile([128, 256], BF16, tag="og", bufs=2)
                        nc.scalar.activation(out=og, in_=o2p, func=AF.Copy)
                        nc.sync.dma_start(out=y_d[c0 + q0:c0 + q0 + 128, :],
                                          in_=og)
    nc.compile()
    return nc


_NC_CACHE = {}
LAST_RESULT = {}


def kernel(**inputs):
    inp = {k: np.ascontiguousarray(np.asarray(v, dtype=np.float32))
           for k, v in inputs.items()}
    if "nc" not in _NC_CACHE:
        _NC_CACHE["nc"] = build_kernel()
    nc = _NC_CACHE["nc"]
    w = _pack_weights(inp)
    in_maps = [_per_core_arrays(inp, w, core // 2, core % 2)
               for core in range(8)]
    kw = {}
    if os.environ.get("KTRACE") == "1":
        kw = {"trace": True, "tmpdir": os.environ.get("KTRACE_DIR") or None}
    res = run_bass_kernel_spmd(nc, in_maps, core_ids=list(range(8)), **kw)
    LAST_RESULT.clear()
    LAST_RESULT["res"] = res
    out = np.empty((B, T, NQ), np.float32)
    for core in range(8):
        b, half = core // 2, core % 2
        y = np.asarray(res.results[core]["y"]).astype(np.float32)
        if half == 0:
            out[b, 0:TA] = y[0:TA]
        else:
            out[b, TA:T] = y[HALO:HALO + (T - TA)]
    return out
